# revision 23
# baseline (speedup 1.0000x reference)
"""Trainium2 Bass kernel for nn_DiffusionModel_56822417326086.

Causal multi-head self-attention block:
    qkv = x @ w_qkv ; split into 8 heads of 64
    e = (q @ k^T) * DH^-0.5 ; causal + key-padding mask ; a = softmax(e)
    o = a @ v ; y = o @ w_out + b_out ; y *= m

Sharding (8 cores, zero collectives):
    core c -> batch b = c // 2, head-quad q = c % 2 (heads 4q..4q+3).
    Each core computes q/k/v for its 4 heads over its whole batch, full
    causal attention for those heads, and the partial output projection
    y_partial = o[heads] @ w_out[head rows].  Host sums the two partials
    per batch (linear unshard), adds b_out, applies the query-side mask.

On-device layout notes:
  - scores are computed TRANSPOSED: sT[key, query] so that the A@V
    contraction (over keys) has keys on the partition dim.
  - softmax denominators come for free as a 65th "ones" column of V.
  - no max-subtraction in softmax: scores are O(1) here, exp is safe.
  - matmuls run as float32r (fp32 data on the fast PE path).
  - all matmul operands live at partition base 0 (base-64 operands fault
    on this runtime), so q/k are stored per-head at partitions 0-63.
  - all 4 heads of one key block share a 2-bank PSUM tile [128, 1024]
    so one ACT Exp op covers them (ACT per-op overhead is ~250 ns).
"""

import numpy as np
import ml_dtypes
from contextlib import ExitStack

B, T, D, H = 4, 2048, 512, 8
DH = D // H
SCALE = DH ** -0.5
NEG = -1.0e30
QC = 512           # query-chunk (free dim of score matmuls)
NQC = T // QC      # 8
KB = 128           # key-block (partition dim of score tiles)

_CACHE = {}


def _build_program():
    import concourse.mybir as mybir
    import concourse.tile as tile
    from concourse import bacc

    f32 = mybir.dt.float32
    f32r = mybir.dt.float32r
    bf16 = mybir.dt.bfloat16
    Exp = mybir.ActivationFunctionType.Exp

    nc = bacc.Bacc("TRN2", target_bir_lowering=False, debug=False)

    xT_d = nc.dram_tensor("xT", [D, T], bf16, kind="ExternalInput").ap()
    wq_d = nc.dram_tensor("wq2", [2, D, 128], bf16, kind="ExternalInput").ap()
    wk_d = nc.dram_tensor("wk2", [2, D, 128], bf16, kind="ExternalInput").ap()
    wv_d = nc.dram_tensor("wv4", [D, 256], bf16, kind="ExternalInput").ap()
    wo_d = nc.dram_tensor("wo4", [256, D], f32r, kind="ExternalInput").ap()
    dm_d = nc.dram_tensor("dm4", [4, 128, 1024], bf16, kind="ExternalInput").ap()
    mk_d = nc.dram_tensor("mkey", [T, 1], f32, kind="ExternalInput").ap()
    y_d = nc.dram_tensor("y", [T, D], f32, kind="ExternalOutput").ap()

    with tile.TileContext(nc) as tc, ExitStack() as ctx:
        consts = ctx.enter_context(tc.tile_pool(name="consts", bufs=1))
        work = ctx.enter_context(tc.tile_pool(name="work", bufs=2))
        ps_big = ctx.enter_context(tc.tile_pool(name="psb", bufs=3, space="PSUM"))
        ps_o = ctx.enter_context(tc.tile_pool(name="pso", bufs=1, space="PSUM"))

        # ---- persistent tiles ----------------------------------------------
        qTa = consts.tile([64, 2, T], f32r)   # [head-in-pair A][pair] q^T
        qTb = consts.tile([64, 2, T], f32r)
        kTa = consts.tile([64, 2, T], f32r)
        kTb = consts.tile([64, 2, T], f32r)
        vsb = consts.tile([128, 16, 4, 65], bf16)
        wo = consts.tile([128, 2, D], f32r)
        mk = consts.tile([128, 16], f32)
        ones41 = consts.tile([128, 4, 1], f32)
        oUA = consts.tile([64, 2, T], f32)
        oUB = consts.tile([64, 2, T], f32)
        sums_stage = consts.tile([8, 1024], f32)
        recips_f = consts.tile([8, 1024], f32)
        recips = consts.tile([8, 1024], f32r)
        ones64 = consts.tile([1, 64], f32)
        ones64r = consts.tile([1, 64], f32r)
        oTn2 = consts.tile([128, 2, T], f32r)

        nc.vector.memset(ones41[:], 1.0)
        nc.vector.memset(ones64[:], 1.0)
        nc.vector.tensor_copy(ones64r[:], ones64[:])
        warm = consts.tile([1, 512], f32r)
        nc.vector.tensor_copy(warm[0:1, 0:64], ones64[:])
        for _ in range(16):
            wps = ps_big.tile([64, 512], f32, tag="scores")
            nc.tensor.matmul(wps[:], ones64r[:], warm[:], start=True, stop=True)
        for p in range(2):
            nc.sync.dma_start(wo[:, p, :], wo_d[p * 128:(p + 1) * 128, :])
        for rc in range(16):
            nc.sync.dma_start(mk[:, rc:rc + 1], mk_d[rc * 128:(rc + 1) * 128, :])

        # ---- qkv projection (phase-scoped SBUF pool) ------------------------
        with tc.tile_pool(name="qkvp", bufs=1) as qp:
            wq = qp.tile([128, 2, 4, 128], bf16)
            wk = qp.tile([128, 2, 4, 128], bf16)
            wv = qp.tile([128, 4, 256], bf16)
            for p in range(2):
                for kc in range(4):
                    nc.sync.dma_start(wq[:, p, kc, :],
                                      wq_d[p, kc * 128:(kc + 1) * 128, :])
                    nc.sync.dma_start(wk[:, p, kc, :],
                                      wk_d[p, kc * 128:(kc + 1) * 128, :])
            for kc in range(4):
                nc.sync.dma_start(wv[:, kc, :], wv_d[kc * 128:(kc + 1) * 128, :])
            xT = qp.tile([128, 4, T], bf16)
            # column-major sub-chunks so the first matmul group's inputs land
            # quickly instead of after the whole 4 MB of x
            for rc4 in range(4):
                for kc in range(4):
                    nc.sync.dma_start(
                        xT[:, kc, rc4 * 512:(rc4 + 1) * 512],
                        xT_d[kc * 128:(kc + 1) * 128, rc4 * 512:(rc4 + 1) * 512])

            for p in range(2):
                for rc4 in range(4):
                    sl = slice(rc4 * 512, (rc4 + 1) * 512)
                    psq = ps_big.tile([128, 512], f32, tag="scores")
                    psk = ps_big.tile([128, 512], f32, tag="scores")
                    for kc in range(4):
                        nc.tensor.matmul(psq[:], wq[:, p, kc, :], xT[:, kc, sl],
                                         start=kc == 0, stop=kc == 3)
                        nc.tensor.matmul(psk[:], wk[:, p, kc, :], xT[:, kc, sl],
                                         start=kc == 0, stop=kc == 3)
                    nc.vector.tensor_copy(qTa[:, p, sl], psq[0:64, :])
                    nc.vector.tensor_copy(kTa[:, p, sl], psk[0:64, :])
                    shq = work.tile([128, 512], f32r, tag="sumscr")
                    nc.vector.tensor_copy(shq[64:128, :], psq[64:128, :])
                    nc.sync.dma_start(qTb[:, p, sl], shq[64:128, :])
                    shk = work.tile([128, 512], f32r, tag="sumscr")
                    nc.vector.tensor_copy(shk[64:128, :], psk[64:128, :])
                    nc.sync.dma_start(kTb[:, p, sl], shk[64:128, :])

            for rc in range(16):
                psv = ps_big.tile([128, 4, 64], f32, tag="scores")
                for kc in range(4):
                    nc.tensor.matmul(psv[:], xT[:, kc, rc * 128:(rc + 1) * 128],
                                     wv[:, kc, :], start=kc == 0, stop=kc == 3)
                nc.vector.tensor_scalar_mul(vsb[:, rc, :, 0:64], psv[:],
                                            mk[:, rc:rc + 1])
                nc.vector.tensor_scalar_mul(vsb[:, rc, :, 64:65], ones41[:],
                                            mk[:, rc:rc + 1])

        # ---- attention (pair-major; 2 heads per 2-bank score tile) ----------
        with tc.tile_pool(name="attp", bufs=1) as ap_, \
             tc.tile_pool(name="exp", bufs=3) as exp_pool:
            dm = ap_.tile([128, 4, 1024], bf16)
            for v_ in range(4):
                nc.sync.dma_start(dm[:, v_, :], dm_d[v_])
            for p in range(2):
                for qc in range(NQC):
                    nkb = 4 * (qc + 1)
                    qsl = slice(qc * QC, (qc + 1) * QC)
                    oA = ps_o.tile([128, 512], f32, tag="oA")
                    oB = ps_o.tile([128, 512], f32, tag="oB")
                    avq = []
                    for kb in range(nkb):
                        ksl = slice(kb * KB, (kb + 1) * KB)
                        sps = ps_big.tile([128, 1024], f32, tag="scores")
                        nc.tensor.matmul(sps[:, 0:512], kTa[:, p, ksl],
                                         qTa[:, p, qsl], start=True, stop=True)
                        nc.tensor.matmul(sps[:, 512:1024], kTb[:, p, ksl],
                                         qTb[:, p, qsl], start=True, stop=True)
                        ex = exp_pool.tile([128, 1024], bf16, tag="exp")
                        nc.scalar.activation(ex[:], sps[:], Exp, scale=SCALE)
                        if kb >= nkb - 4:
                            nc.vector.tensor_mul(ex[:], ex[:],
                                                 dm[:, kb - (nkb - 4), :])
                        avq.append((kb, ex))
                        if len(avq) > 1:
                            _em(nc, avq.pop(0), oA, oB, vsb, p, nkb)
                    _em(nc, avq.pop(0), oA, oB, vsb, p, nkb)

                    scr = work.tile([128, 1024], f32, tag="sumscr")
                    nc.vector.tensor_copy(scr[64:65, 0:512], oA[64:65, :])
                    nc.vector.tensor_copy(scr[64:65, 512:1024], oB[64:65, :])
                    idx = p * 4 + qc
                    nc.sync.dma_start(sums_stage[idx:idx + 1, :], scr[64:65, :])
                    nc.vector.tensor_copy(oUA[:, p, qsl], oA[0:64, :])
                    nc.vector.tensor_copy(oUB[:, p, qsl], oB[0:64, :])

        # ---- normalize + output projection, interleaved per query chunk -----
        nc.vector.reciprocal(recips_f[:], sums_stage[:])
        nc.vector.tensor_copy(recips[:], recips_f[:])
        for qc in range(NQC):
            qsl = slice(qc * QC, (qc + 1) * QC)
            for p in range(2):
                idx = p * 4 + qc
                rec = work.tile([1, 1024], f32r, tag="rec")
                nc.sync.dma_start(rec[:], recips[idx:idx + 1, :])
                bcA = ps_big.tile([64, 512], f32, tag="scores")
                nc.tensor.matmul(bcA[:], ones64r[:], rec[0:1, 0:512],
                                 start=True, stop=True)
                nc.vector.tensor_mul(oTn2[0:64, p, qsl], oUA[:, p, qsl], bcA[:])
                bcB = ps_big.tile([64, 512], f32, tag="scores")
                nc.tensor.matmul(bcB[:], ones64r[:], rec[0:1, 512:1024],
                                 start=True, stop=True)
                scrB = work.tile([64, 512], f32r, tag="scrB")
                nc.vector.tensor_mul(scrB[:], oUB[:, p, qsl], bcB[:])
                # partition shift 0-63 -> 64-127 (DVE lanes are partition-locked)
                nc.sync.dma_start(oTn2[64:128, p, qsl], scrB[:])
            for rc in range(4 * qc, 4 * qc + 4):
                rsl = slice(rc * 128, (rc + 1) * 128)
                psy = ps_big.tile([128, 512], f32, tag="scores")
                for p in range(2):
                    nc.tensor.matmul(psy[:], oTn2[:, p, rsl], wo[:, p, :],
                                     start=p == 0, stop=p == 1)
                yt = work.tile([128, 512], f32, tag="ysb")
                nc.vector.tensor_copy(yt[:], psy[:])
                nc.sync.dma_start(y_d[rsl, :], yt[:])

    nc.compile()
    return nc


def _em(nc, item, oA, oB, vsb, p, nkb):
    """Emit the deferred A@V accumulations for one key block (one pair)."""
    kb, ex = item
    nc.tensor.matmul(oA[0:65, :], vsb[:, kb, 2 * p, :], ex[:, 0:512],
                     start=kb == 0, stop=kb == nkb - 1)
    nc.tensor.matmul(oB[0:65, :], vsb[:, kb, 2 * p + 1, :], ex[:, 512:1024],
                     start=kb == 0, stop=kb == nkb - 1)


def _diag_masks():
    i = np.arange(QC)[None, :]
    j = np.arange(KB)[:, None]
    out = []
    for v in range(4):
        mv = np.where(i >= j + v * KB, 1.0, 0.0).astype(np.float32)
        out.append(np.tile(mv, (1, 2)).copy())
    return out


def _prep_inputs(x, m, w_qkv, w_out):
    """Per-core input maps for SPMD dispatch."""
    dm4 = np.stack(_diag_masks()).astype(ml_dtypes.bfloat16)
    wq_full = w_qkv[:, 0:D]
    wk_full = w_qkv[:, D:2 * D]
    wv_full = w_qkv[:, 2 * D:3 * D]
    in_maps = []
    for c in range(8):
        b, q = c // 2, c % 2
        hsl = slice(4 * q * DH, (4 * q + 4) * DH)
        wq2 = np.stack([
            np.concatenate([wq_full[:, (4 * q + 2 * p) * DH:(4 * q + 2 * p + 1) * DH],
                            wq_full[:, (4 * q + 2 * p + 1) * DH:(4 * q + 2 * p + 2) * DH]],
                           axis=1)
            for p in range(2)])
        wk2 = np.stack([
            np.concatenate([wk_full[:, (4 * q + 2 * p) * DH:(4 * q + 2 * p + 1) * DH],
                            wk_full[:, (4 * q + 2 * p + 1) * DH:(4 * q + 2 * p + 2) * DH]],
                           axis=1)
            for p in range(2)])
        in_maps.append({
            "xT": np.ascontiguousarray(x[b].T).astype(ml_dtypes.bfloat16),
            "wq2": np.ascontiguousarray(wq2).astype(ml_dtypes.bfloat16),
            "wk2": np.ascontiguousarray(wk2).astype(ml_dtypes.bfloat16),
            "wv4": np.ascontiguousarray(wv_full[:, hsl]).astype(ml_dtypes.bfloat16),
            "wo4": np.ascontiguousarray(w_out[hsl, :]).astype(np.float32),
            "dm4": dm4,
            "mkey": np.ascontiguousarray((m[b] != 0).astype(np.float32)[:, None]),
        })
    return in_maps


def _execute(inputs, trace=False):
    from concourse.bass_utils import run_bass_kernel_spmd

    if "nc" not in _CACHE:
        _CACHE["nc"] = _build_program()
    nc = _CACHE["nc"]

    x = np.asarray(inputs["x"], np.float32)
    m = np.asarray(inputs["m"], np.float32)
    w_qkv = np.asarray(inputs["w_qkv"], np.float32)
    w_out = np.asarray(inputs["w_out"], np.float32)
    b_out = np.asarray(inputs["b_out"], np.float32)

    in_maps = _prep_inputs(x, m, w_qkv, w_out)
    res = run_bass_kernel_spmd(nc, in_maps, core_ids=list(range(8)), trace=trace)

    y = np.empty((B, T, D), np.float32)
    for b in range(B):
        y[b] = res.results[2 * b]["y"] + res.results[2 * b + 1]["y"]
    y += b_out[None, None, :]
    y *= m[..., None]
    return y, res


def kernel(**inputs) -> np.ndarray:
    y, _ = _execute(inputs, trace=False)
    return y


# revision 24
# speedup vs baseline: 1.2259x; 1.2259x over previous
"""Trainium2 Bass kernel for nn_DiffusionModel_56822417326086.

Causal multi-head self-attention block:
    qkv = x @ w_qkv ; split into 8 heads of 64
    e = (q @ k^T) * DH^-0.5 ; causal + key-padding mask ; a = softmax(e)
    o = a @ v ; y = o @ w_out + b_out ; y *= m

Sharding (8 cores, zero collectives):
    core c -> batch b = c // 2, head-quad q = c % 2 (heads 4q..4q+3).
    Each core computes q/k/v for its 4 heads over its whole batch, full
    causal attention for those heads, and the partial output projection
    y_partial = o[heads] @ w_out[head rows].  Host sums the two partials
    per batch (linear unshard), adds b_out, applies the query-side mask.

On-device layout notes:
  - scores are computed TRANSPOSED: sT[key, query] so that the A@V
    contraction (over keys) has keys on the partition dim.
  - softmax denominators come for free as a 65th "ones" column of V.
  - no max-subtraction in softmax: scores are O(1) here, exp is safe.
  - matmuls run as float32r (fp32 data on the fast PE path).
  - all matmul operands live at partition base 0 (base-64 operands fault
    on this runtime), so q/k are stored per-head at partitions 0-63.
  - all 4 heads of one key block share a 2-bank PSUM tile [128, 1024]
    so one ACT Exp op covers them (ACT per-op overhead is ~250 ns).
"""

import numpy as np
import ml_dtypes
from contextlib import ExitStack

B, T, D, H = 4, 2048, 512, 8
DH = D // H
SCALE = DH ** -0.5
NEG = -1.0e30
QC = 512           # query-chunk (free dim of score matmuls)
NQC = T // QC      # 8
KB = 128           # key-block (partition dim of score tiles)

_CACHE = {}


def _build_program():
    import concourse.mybir as mybir
    import concourse.tile as tile
    from concourse import bacc

    f32 = mybir.dt.float32
    f32r = mybir.dt.float32r
    bf16 = mybir.dt.bfloat16
    Exp = mybir.ActivationFunctionType.Exp

    nc = bacc.Bacc("TRN2", target_bir_lowering=False, debug=False)

    xT_d = nc.dram_tensor("xT", [D, T], bf16, kind="ExternalInput").ap()
    wq_d = nc.dram_tensor("wq2", [2, D, 128], bf16, kind="ExternalInput").ap()
    wk_d = nc.dram_tensor("wk2", [2, D, 128], bf16, kind="ExternalInput").ap()
    wv_d = nc.dram_tensor("wv4", [D, 256], bf16, kind="ExternalInput").ap()
    wo_d = nc.dram_tensor("wo4", [256, D], f32r, kind="ExternalInput").ap()
    dm_d = nc.dram_tensor("dm4", [4, 128, 1024], bf16, kind="ExternalInput").ap()
    mk_d = nc.dram_tensor("mkey", [T, 1], f32, kind="ExternalInput").ap()
    y_d = nc.dram_tensor("y", [T, D], f32, kind="ExternalOutput").ap()

    with tile.TileContext(nc) as tc, ExitStack() as ctx:
        consts = ctx.enter_context(tc.tile_pool(name="consts", bufs=1))
        work = ctx.enter_context(tc.tile_pool(name="work", bufs=2))
        ps_big = ctx.enter_context(tc.tile_pool(name="psb", bufs=3, space="PSUM"))
        ps_o = ctx.enter_context(tc.tile_pool(name="pso", bufs=1, space="PSUM"))

        # ---- persistent tiles ----------------------------------------------
        # packed q^T/k^T: partitions 0-63 = head A of pair, 64-127 = head B
        qT2 = consts.tile([128, 2, T], f32r)
        kT2 = consts.tile([128, 2, T], f32r)
        vsb = consts.tile([128, 16, 4, 65], bf16)
        wo = consts.tile([128, 2, D], f32r)
        mk = consts.tile([128, 16], f32)
        ones41 = consts.tile([128, 4, 1], f32)
        oUA = consts.tile([64, 2, T], f32)
        oUB = consts.tile([64, 2, T], f32)
        sums_stage = consts.tile([8, 1024], f32)
        recips_f = consts.tile([8, 1024], f32)
        recips = consts.tile([8, 1024], f32r)
        ones64 = consts.tile([1, 64], f32)
        ones64r = consts.tile([1, 64], f32r)
        oTn2 = consts.tile([128, 2, T], f32r)

        nc.vector.memset(ones41[:], 1.0)
        nc.vector.memset(ones64[:], 1.0)
        nc.vector.tensor_copy(ones64r[:], ones64[:])
        warm = consts.tile([1, 512], f32r)
        nc.vector.tensor_copy(warm[0:1, 0:64], ones64[:])
        for _ in range(16):
            wps = ps_big.tile([64, 512], f32, tag="scores")
            nc.tensor.matmul(wps[:], ones64r[:], warm[:], start=True, stop=True)
        for p in range(2):
            nc.sync.dma_start(wo[:, p, :], wo_d[p * 128:(p + 1) * 128, :])
        for rc in range(16):
            nc.sync.dma_start(mk[:, rc:rc + 1], mk_d[rc * 128:(rc + 1) * 128, :])

        # ---- qkv projection (phase-scoped SBUF pool) ------------------------
        with tc.tile_pool(name="qkvp", bufs=1) as qp:
            wq = qp.tile([128, 2, 4, 128], bf16)
            wk = qp.tile([128, 2, 4, 128], bf16)
            wv = qp.tile([128, 4, 256], bf16)
            for p in range(2):
                for kc in range(4):
                    nc.sync.dma_start(wq[:, p, kc, :],
                                      wq_d[p, kc * 128:(kc + 1) * 128, :])
                    nc.sync.dma_start(wk[:, p, kc, :],
                                      wk_d[p, kc * 128:(kc + 1) * 128, :])
            for kc in range(4):
                nc.sync.dma_start(wv[:, kc, :], wv_d[kc * 128:(kc + 1) * 128, :])
            xT = qp.tile([128, 4, T], bf16)
            # column-major sub-chunks so the first matmul group's inputs land
            # quickly instead of after the whole 4 MB of x
            for rc4 in range(4):
                for kc in range(4):
                    nc.sync.dma_start(
                        xT[:, kc, rc4 * 512:(rc4 + 1) * 512],
                        xT_d[kc * 128:(kc + 1) * 128, rc4 * 512:(rc4 + 1) * 512])

            for p in range(2):
                for rc4 in range(4):
                    sl = slice(rc4 * 512, (rc4 + 1) * 512)
                    psq = ps_big.tile([128, 512], f32, tag="scores")
                    psk = ps_big.tile([128, 512], f32, tag="scores")
                    for kc in range(4):
                        nc.tensor.matmul(psq[:], wq[:, p, kc, :], xT[:, kc, sl],
                                         start=kc == 0, stop=kc == 3)
                        nc.tensor.matmul(psk[:], wk[:, p, kc, :], xT[:, kc, sl],
                                         start=kc == 0, stop=kc == 3)
                    nc.vector.tensor_copy(qT2[:, p, sl], psq[:])
                    nc.vector.tensor_copy(kT2[:, p, sl], psk[:])

            for rc in range(16):
                psv = ps_big.tile([128, 4, 64], f32, tag="scores")
                for kc in range(4):
                    nc.tensor.matmul(psv[:], xT[:, kc, rc * 128:(rc + 1) * 128],
                                     wv[:, kc, :], start=kc == 0, stop=kc == 3)
                nc.vector.tensor_scalar_mul(vsb[:, rc, :, 0:64], psv[:],
                                            mk[:, rc:rc + 1])
                nc.vector.tensor_scalar_mul(vsb[:, rc, :, 64:65], ones41[:],
                                            mk[:, rc:rc + 1])

        # ---- attention (pair-major; 2 heads per 2-bank score tile) ----------
        with tc.tile_pool(name="attp", bufs=1) as ap_, \
             tc.tile_pool(name="exp", bufs=3) as exp_pool:
            dm = ap_.tile([128, 4, 1024], bf16)
            for v_ in range(4):
                nc.sync.dma_start(dm[:, v_, :], dm_d[v_])
            for p in range(2):
                for qc in range(NQC):
                    nkb = 4 * (qc + 1)
                    qsl = slice(qc * QC, (qc + 1) * QC)
                    oA = ps_o.tile([128, 512], f32, tag="oA")
                    oB = ps_o.tile([128, 512], f32, tag="oB")
                    avq = []
                    for kb in range(nkb):
                        ksl = slice(kb * KB, (kb + 1) * KB)
                        sps = ps_big.tile([128, 1024], f32, tag="scores")
                        # row-tiled pair: K=64 each, concurrent in the array;
                        # outputs land in DIFFERENT PSUM banks (same-bank
                        # dual-write faults the exec unit)
                        nc.tensor.matmul(sps[:, 0:512], kT2[0:64, p, ksl],
                                         qT2[0:64, p, qsl], start=True, stop=True,
                                         tile_position=(0, 0))
                        nc.tensor.matmul(sps[:, 512:1024], kT2[64:128, p, ksl],
                                         qT2[64:128, p, qsl], start=True, stop=True,
                                         tile_position=(64, 0))
                        ex = exp_pool.tile([128, 1024], bf16, tag="exp")
                        nc.scalar.activation(ex[:], sps[:], Exp, scale=SCALE)
                        if kb >= nkb - 4:
                            nc.vector.tensor_mul(ex[:], ex[:],
                                                 dm[:, kb - (nkb - 4), :])
                        avq.append((kb, ex))
                        if len(avq) > 1:
                            _em(nc, avq.pop(0), oA, oB, vsb, p, nkb)
                    _em(nc, avq.pop(0), oA, oB, vsb, p, nkb)

                    scr = work.tile([128, 1024], f32, tag="sumscr")
                    nc.vector.tensor_copy(scr[64:65, 0:512], oA[64:65, :])
                    nc.vector.tensor_copy(scr[64:65, 512:1024], oB[64:65, :])
                    idx = p * 4 + qc
                    nc.sync.dma_start(sums_stage[idx:idx + 1, :], scr[64:65, :])
                    nc.vector.tensor_copy(oUA[:, p, qsl], oA[0:64, :])
                    nc.vector.tensor_copy(oUB[:, p, qsl], oB[0:64, :])

        # ---- normalize + output projection, interleaved per query chunk -----
        nc.vector.reciprocal(recips_f[:], sums_stage[:])
        nc.vector.tensor_copy(recips[:], recips_f[:])
        for qc in range(NQC):
            qsl = slice(qc * QC, (qc + 1) * QC)
            for p in range(2):
                idx = p * 4 + qc
                rec = work.tile([1, 1024], f32r, tag="rec")
                nc.sync.dma_start(rec[:], recips[idx:idx + 1, :])
                bcA = ps_big.tile([64, 512], f32, tag="scores")
                nc.tensor.matmul(bcA[:], ones64r[:], rec[0:1, 0:512],
                                 start=True, stop=True)
                nc.vector.tensor_mul(oTn2[0:64, p, qsl], oUA[:, p, qsl], bcA[:])
                bcB = ps_big.tile([64, 512], f32, tag="scores")
                nc.tensor.matmul(bcB[:], ones64r[:], rec[0:1, 512:1024],
                                 start=True, stop=True)
                scrB = work.tile([64, 512], f32r, tag="scrB")
                nc.vector.tensor_mul(scrB[:], oUB[:, p, qsl], bcB[:])
                # partition shift 0-63 -> 64-127 (DVE lanes are partition-locked)
                nc.sync.dma_start(oTn2[64:128, p, qsl], scrB[:])
            for rc in range(4 * qc, 4 * qc + 4):
                rsl = slice(rc * 128, (rc + 1) * 128)
                psy = ps_big.tile([128, 512], f32, tag="scores")
                for p in range(2):
                    nc.tensor.matmul(psy[:], oTn2[:, p, rsl], wo[:, p, :],
                                     start=p == 0, stop=p == 1)
                yt = work.tile([128, 512], f32, tag="ysb")
                nc.vector.tensor_copy(yt[:], psy[:])
                nc.sync.dma_start(y_d[rsl, :], yt[:])

    nc.compile()
    return nc


def _em(nc, item, oA, oB, vsb, p, nkb):
    """Emit the deferred A@V accumulations for one key block (one pair)."""
    kb, ex = item
    nc.tensor.matmul(oA[0:65, :], vsb[:, kb, 2 * p, :], ex[:, 0:512],
                     start=kb == 0, stop=kb == nkb - 1)
    nc.tensor.matmul(oB[0:65, :], vsb[:, kb, 2 * p + 1, :], ex[:, 512:1024],
                     start=kb == 0, stop=kb == nkb - 1)


def _diag_masks():
    i = np.arange(QC)[None, :]
    j = np.arange(KB)[:, None]
    out = []
    for v in range(4):
        mv = np.where(i >= j + v * KB, 1.0, 0.0).astype(np.float32)
        out.append(np.tile(mv, (1, 2)).copy())
    return out


def _prep_inputs(x, m, w_qkv, w_out):
    """Per-core input maps for SPMD dispatch."""
    dm4 = np.stack(_diag_masks()).astype(ml_dtypes.bfloat16)
    wq_full = w_qkv[:, 0:D]
    wk_full = w_qkv[:, D:2 * D]
    wv_full = w_qkv[:, 2 * D:3 * D]
    in_maps = []
    for c in range(8):
        b, q = c // 2, c % 2
        hsl = slice(4 * q * DH, (4 * q + 4) * DH)
        wq2 = np.stack([
            np.concatenate([wq_full[:, (4 * q + 2 * p) * DH:(4 * q + 2 * p + 1) * DH],
                            wq_full[:, (4 * q + 2 * p + 1) * DH:(4 * q + 2 * p + 2) * DH]],
                           axis=1)
            for p in range(2)])
        wk2 = np.stack([
            np.concatenate([wk_full[:, (4 * q + 2 * p) * DH:(4 * q + 2 * p + 1) * DH],
                            wk_full[:, (4 * q + 2 * p + 1) * DH:(4 * q + 2 * p + 2) * DH]],
                           axis=1)
            for p in range(2)])
        in_maps.append({
            "xT": np.ascontiguousarray(x[b].T).astype(ml_dtypes.bfloat16),
            "wq2": np.ascontiguousarray(wq2).astype(ml_dtypes.bfloat16),
            "wk2": np.ascontiguousarray(wk2).astype(ml_dtypes.bfloat16),
            "wv4": np.ascontiguousarray(wv_full[:, hsl]).astype(ml_dtypes.bfloat16),
            "wo4": np.ascontiguousarray(w_out[hsl, :]).astype(np.float32),
            "dm4": dm4,
            "mkey": np.ascontiguousarray((m[b] != 0).astype(np.float32)[:, None]),
        })
    return in_maps


def _execute(inputs, trace=False):
    from concourse.bass_utils import run_bass_kernel_spmd

    if "nc" not in _CACHE:
        _CACHE["nc"] = _build_program()
    nc = _CACHE["nc"]

    x = np.asarray(inputs["x"], np.float32)
    m = np.asarray(inputs["m"], np.float32)
    w_qkv = np.asarray(inputs["w_qkv"], np.float32)
    w_out = np.asarray(inputs["w_out"], np.float32)
    b_out = np.asarray(inputs["b_out"], np.float32)

    in_maps = _prep_inputs(x, m, w_qkv, w_out)
    res = run_bass_kernel_spmd(nc, in_maps, core_ids=list(range(8)), trace=trace)

    y = np.empty((B, T, D), np.float32)
    for b in range(B):
        y[b] = res.results[2 * b]["y"] + res.results[2 * b + 1]["y"]
    y += b_out[None, None, :]
    y *= m[..., None]
    return y, res


def kernel(**inputs) -> np.ndarray:
    y, _ = _execute(inputs, trace=False)
    return y


# revision 27
# speedup vs baseline: 1.2429x; 1.0138x over previous
"""Trainium2 Bass kernel for nn_DiffusionModel_56822417326086.

Causal multi-head self-attention block:
    qkv = x @ w_qkv ; split into 8 heads of 64
    e = (q @ k^T) * DH^-0.5 ; causal + key-padding mask ; a = softmax(e)
    o = a @ v ; y = o @ w_out + b_out ; y *= m

Sharding (8 cores, zero collectives):
    core c -> batch b = c // 2, head-quad q = c % 2 (heads 4q..4q+3).
    Each core computes q/k/v for its 4 heads over its whole batch, full
    causal attention for those heads, and the partial output projection
    y_partial = o[heads] @ w_out[head rows].  Host sums the two partials
    per batch (linear unshard), adds b_out, applies the query-side mask.

On-device layout notes:
  - scores are computed TRANSPOSED: sT[key, query] so that the A@V
    contraction (over keys) has keys on the partition dim.
  - softmax denominators come for free as a 65th "ones" column of V.
  - no max-subtraction in softmax: scores are O(1) here, exp is safe.
  - matmuls run as float32r (fp32 data on the fast PE path).
  - all matmul operands live at partition base 0 (base-64 operands fault
    on this runtime), so q/k are stored per-head at partitions 0-63.
  - all 4 heads of one key block share a 2-bank PSUM tile [128, 1024]
    so one ACT Exp op covers them (ACT per-op overhead is ~250 ns).
"""

import numpy as np
import ml_dtypes
from contextlib import ExitStack

B, T, D, H = 4, 2048, 512, 8
DH = D // H
SCALE = DH ** -0.5
NEG = -1.0e30
QC = 512           # query-chunk (free dim of score matmuls)
NQC = T // QC      # 8
KB = 128           # key-block (partition dim of score tiles)

_CACHE = {}


def _build_program():
    import concourse.mybir as mybir
    import concourse.tile as tile
    from concourse import bacc

    f32 = mybir.dt.float32
    f32r = mybir.dt.float32r
    bf16 = mybir.dt.bfloat16
    Exp = mybir.ActivationFunctionType.Exp

    nc = bacc.Bacc("TRN2", target_bir_lowering=False, debug=False)

    xT_d = nc.dram_tensor("xT", [D, T], bf16, kind="ExternalInput").ap()
    wq_d = nc.dram_tensor("wq2", [2, D, 128], bf16, kind="ExternalInput").ap()
    wk_d = nc.dram_tensor("wk2", [2, D, 128], bf16, kind="ExternalInput").ap()
    wv_d = nc.dram_tensor("wv4", [D, 256], bf16, kind="ExternalInput").ap()
    wo_d = nc.dram_tensor("wo4", [256, D], f32r, kind="ExternalInput").ap()
    dm_d = nc.dram_tensor("dm4", [4, 128, 1024], bf16, kind="ExternalInput").ap()
    mk_d = nc.dram_tensor("mkey", [T, 1], f32, kind="ExternalInput").ap()
    y_d = nc.dram_tensor("y", [T, D], f32, kind="ExternalOutput").ap()

    with tile.TileContext(nc) as tc, ExitStack() as ctx:
        consts = ctx.enter_context(tc.tile_pool(name="consts", bufs=1))
        work = ctx.enter_context(tc.tile_pool(name="work", bufs=2))
        ps_big = ctx.enter_context(tc.tile_pool(name="psb", bufs=3, space="PSUM"))
        ps_o = ctx.enter_context(tc.tile_pool(name="pso", bufs=1, space="PSUM"))

        # ---- persistent tiles ----------------------------------------------
        # packed q^T/k^T: partitions 0-63 = head A of pair, 64-127 = head B
        qT2 = consts.tile([128, 2, T], f32r)
        kT2 = consts.tile([128, 2, T], f32r)
        vsb = consts.tile([128, 16, 4, 65], bf16)
        wo = consts.tile([128, 2, D], f32r)
        mk = consts.tile([128, 16], f32)
        ones41 = consts.tile([128, 4, 1], f32)
        oUA = consts.tile([64, 2, T], f32)
        oUB = consts.tile([64, 2, T], f32)
        sums_stage = consts.tile([36, 1024], f32)   # p0 rows 0-3, p1 rows 32-35
        recips_f = consts.tile([36, 1024], f32)
        recips = consts.tile([36, 1024], f32r)
        ones64 = consts.tile([1, 64], f32)
        ones64r = consts.tile([1, 64], f32r)
        oTn2 = consts.tile([128, 2, T], f32r)

        nc.vector.memset(ones41[:], 1.0)
        nc.vector.memset(ones64[:], 1.0)
        nc.vector.tensor_copy(ones64r[:], ones64[:])
        warm = consts.tile([1, 512], f32r)
        nc.vector.tensor_copy(warm[0:1, 0:64], ones64[:])
        for _ in range(16):
            wps = ps_big.tile([64, 512], f32, tag="scores")
            nc.tensor.matmul(wps[:], ones64r[:], warm[:], start=True, stop=True)
        for p in range(2):
            nc.sync.dma_start(wo[:, p, :], wo_d[p * 128:(p + 1) * 128, :])
        for rc in range(16):
            nc.sync.dma_start(mk[:, rc:rc + 1], mk_d[rc * 128:(rc + 1) * 128, :])

        # ---- qkv projection (phase-scoped SBUF pool) ------------------------
        with tc.tile_pool(name="qkvp", bufs=1) as qp:
            wq = qp.tile([128, 2, 4, 128], bf16)
            wk = qp.tile([128, 2, 4, 128], bf16)
            wv = qp.tile([128, 4, 256], bf16)
            for p in range(2):
                for kc in range(4):
                    nc.gpsimd.dma_start(wq[:, p, kc, :],
                                        wq_d[p, kc * 128:(kc + 1) * 128, :])
                    nc.scalar.dma_start(wk[:, p, kc, :],
                                        wk_d[p, kc * 128:(kc + 1) * 128, :])
            for kc in range(4):
                nc.sync.dma_start(wv[:, kc, :], wv_d[kc * 128:(kc + 1) * 128, :])
            xT = qp.tile([128, 4, T], bf16)
            # column-major sub-chunks so the first matmul group's inputs land
            # quickly instead of after the whole 4 MB of x
            _eng = [nc.sync, nc.gpsimd, nc.scalar, nc.gpsimd]
            for rc4 in range(4):
                for kc in range(4):
                    _eng[kc].dma_start(
                        xT[:, kc, rc4 * 512:(rc4 + 1) * 512],
                        xT_d[kc * 128:(kc + 1) * 128, rc4 * 512:(rc4 + 1) * 512])

            for p in range(2):
                for rc4 in range(4):
                    sl = slice(rc4 * 512, (rc4 + 1) * 512)
                    psq = ps_big.tile([128, 512], f32, tag="scores")
                    psk = ps_big.tile([128, 512], f32, tag="scores")
                    for kc in range(4):
                        nc.tensor.matmul(psq[:], wq[:, p, kc, :], xT[:, kc, sl],
                                         start=kc == 0, stop=kc == 3)
                        nc.tensor.matmul(psk[:], wk[:, p, kc, :], xT[:, kc, sl],
                                         start=kc == 0, stop=kc == 3)
                    nc.vector.tensor_copy(qT2[:, p, sl], psq[:])
                    nc.vector.tensor_copy(kT2[:, p, sl], psk[:])

            for rc in range(16):
                psv = ps_big.tile([128, 4, 64], f32, tag="scores")
                for kc in range(4):
                    nc.tensor.matmul(psv[:], xT[:, kc, rc * 128:(rc + 1) * 128],
                                     wv[:, kc, :], start=kc == 0, stop=kc == 3)
                nc.vector.tensor_scalar_mul(vsb[:, rc, :, 0:64], psv[:],
                                            mk[:, rc:rc + 1])
                nc.vector.tensor_scalar_mul(vsb[:, rc, :, 64:65], ones41[:],
                                            mk[:, rc:rc + 1])

        # ---- attention (pair-major; 2 heads per 2-bank score tile) ----------
        with tc.tile_pool(name="attp", bufs=1) as ap_, \
             tc.tile_pool(name="exp", bufs=4) as exp_pool:
            dm = ap_.tile([128, 4, 1024], bf16)
            for v_ in range(4):
                nc.sync.dma_start(dm[:, v_, :], dm_d[v_])
            for p in range(2):
                for qc in range(NQC):
                    nkb = 4 * (qc + 1)
                    qsl = slice(qc * QC, (qc + 1) * QC)
                    oA = ps_o.tile([128, 512], f32, tag="oA")
                    oB = ps_o.tile([128, 512], f32, tag="oB")
                    avq = []
                    for kb in range(nkb):
                        ksl = slice(kb * KB, (kb + 1) * KB)
                        sps = ps_big.tile([128, 1024], f32, tag="scores")
                        # row-tiled pair: K=64 each, concurrent in the array;
                        # outputs land in DIFFERENT PSUM banks (same-bank
                        # dual-write faults the exec unit)
                        nc.tensor.matmul(sps[:, 0:512], kT2[0:64, p, ksl],
                                         qT2[0:64, p, qsl], start=True, stop=True,
                                         tile_position=(0, 0))
                        nc.tensor.matmul(sps[:, 512:1024], kT2[64:128, p, ksl],
                                         qT2[64:128, p, qsl], start=True, stop=True,
                                         tile_position=(64, 0))
                        ex = exp_pool.tile([128, 1024], bf16, tag="exp")
                        nc.scalar.activation(ex[:], sps[:], Exp, scale=SCALE)
                        if kb >= nkb - 4:
                            nc.vector.tensor_mul(ex[:], ex[:],
                                                 dm[:, kb - (nkb - 4), :])
                        avq.append((kb, ex))
                        if len(avq) > 1:
                            _em(nc, avq.pop(0), oA, oB, vsb, p, nkb)
                    _em(nc, avq.pop(0), oA, oB, vsb, p, nkb)

                    scr = work.tile([128, 1024], f32, tag="sumscr")
                    nc.vector.tensor_copy(scr[64:65, 0:512], oA[64:65, :])
                    nc.vector.tensor_copy(scr[64:65, 512:1024], oB[64:65, :])
                    idx = p * 32 + qc
                    nc.sync.dma_start(sums_stage[idx:idx + 1, :], scr[64:65, :])
                    nc.vector.tensor_copy(oUA[:, p, qsl], oA[0:64, :])
                    nc.vector.tensor_copy(oUB[:, p, qsl], oB[0:64, :])

        # ---- normalize + output projection ---------------------------------
        # reciprocal is split per pair: p0's normalization only depends on
        # p0's sums, so the scheduler can run it under p1's attention.
        for p in range(2):
            nc.vector.reciprocal(recips_f[p * 32:p * 32 + 4, :],
                                 sums_stage[p * 32:p * 32 + 4, :])
            nc.vector.tensor_copy(recips[p * 32:p * 32 + 4, :],
                                  recips_f[p * 32:p * 32 + 4, :])
        for qc in range(NQC):
            qsl = slice(qc * QC, (qc + 1) * QC)
            for p in range(2):
                idx = p * 32 + qc
                rec = work.tile([1, 1024], f32r, tag="rec")
                nc.sync.dma_start(rec[:], recips[idx:idx + 1, :])
                bcA = ps_big.tile([64, 512], f32, tag="scores")
                nc.tensor.matmul(bcA[:], ones64r[:], rec[0:1, 0:512],
                                 start=True, stop=True)
                nc.vector.tensor_mul(oTn2[0:64, p, qsl], oUA[:, p, qsl], bcA[:])
                bcB = ps_big.tile([64, 512], f32, tag="scores")
                nc.tensor.matmul(bcB[:], ones64r[:], rec[0:1, 512:1024],
                                 start=True, stop=True)
                scrB = work.tile([64, 512], f32r, tag="scrB")
                nc.vector.tensor_mul(scrB[:], oUB[:, p, qsl], bcB[:])
                # partition shift 0-63 -> 64-127 (DVE lanes are partition-locked)
                nc.sync.dma_start(oTn2[64:128, p, qsl], scrB[:])
            for rc in range(4 * qc, 4 * qc + 4):
                rsl = slice(rc * 128, (rc + 1) * 128)
                psy = ps_big.tile([128, 512], f32, tag="scores")
                for p in range(2):
                    nc.tensor.matmul(psy[:], oTn2[:, p, rsl], wo[:, p, :],
                                     start=p == 0, stop=p == 1)
                yt = work.tile([128, 512], f32, tag="ysb")
                nc.vector.tensor_copy(yt[:], psy[:])
                nc.sync.dma_start(y_d[rsl, :], yt[:])

    nc.compile()
    return nc


def _em(nc, item, oA, oB, vsb, p, nkb):
    """Emit the deferred A@V accumulations for one key block (one pair)."""
    kb, ex = item
    nc.tensor.matmul(oA[0:65, :], vsb[:, kb, 2 * p, :], ex[:, 0:512],
                     start=kb == 0, stop=kb == nkb - 1)
    nc.tensor.matmul(oB[0:65, :], vsb[:, kb, 2 * p + 1, :], ex[:, 512:1024],
                     start=kb == 0, stop=kb == nkb - 1)


def _diag_masks():
    i = np.arange(QC)[None, :]
    j = np.arange(KB)[:, None]
    out = []
    for v in range(4):
        mv = np.where(i >= j + v * KB, 1.0, 0.0).astype(np.float32)
        out.append(np.tile(mv, (1, 2)).copy())
    return out


def _prep_inputs(x, m, w_qkv, w_out):
    """Per-core input maps for SPMD dispatch."""
    dm4 = np.stack(_diag_masks()).astype(ml_dtypes.bfloat16)
    wq_full = w_qkv[:, 0:D]
    wk_full = w_qkv[:, D:2 * D]
    wv_full = w_qkv[:, 2 * D:3 * D]
    in_maps = []
    for c in range(8):
        b, q = c // 2, c % 2
        hsl = slice(4 * q * DH, (4 * q + 4) * DH)
        wq2 = np.stack([
            np.concatenate([wq_full[:, (4 * q + 2 * p) * DH:(4 * q + 2 * p + 1) * DH],
                            wq_full[:, (4 * q + 2 * p + 1) * DH:(4 * q + 2 * p + 2) * DH]],
                           axis=1)
            for p in range(2)])
        wk2 = np.stack([
            np.concatenate([wk_full[:, (4 * q + 2 * p) * DH:(4 * q + 2 * p + 1) * DH],
                            wk_full[:, (4 * q + 2 * p + 1) * DH:(4 * q + 2 * p + 2) * DH]],
                           axis=1)
            for p in range(2)])
        in_maps.append({
            "xT": np.ascontiguousarray(x[b].T).astype(ml_dtypes.bfloat16),
            "wq2": np.ascontiguousarray(wq2).astype(ml_dtypes.bfloat16),
            "wk2": np.ascontiguousarray(wk2).astype(ml_dtypes.bfloat16),
            "wv4": np.ascontiguousarray(wv_full[:, hsl]).astype(ml_dtypes.bfloat16),
            "wo4": np.ascontiguousarray(w_out[hsl, :]).astype(np.float32),
            "dm4": dm4,
            "mkey": np.ascontiguousarray((m[b] != 0).astype(np.float32)[:, None]),
        })
    return in_maps


def _execute(inputs, trace=False):
    from concourse.bass_utils import run_bass_kernel_spmd

    if "nc" not in _CACHE:
        _CACHE["nc"] = _build_program()
    nc = _CACHE["nc"]

    x = np.asarray(inputs["x"], np.float32)
    m = np.asarray(inputs["m"], np.float32)
    w_qkv = np.asarray(inputs["w_qkv"], np.float32)
    w_out = np.asarray(inputs["w_out"], np.float32)
    b_out = np.asarray(inputs["b_out"], np.float32)

    in_maps = _prep_inputs(x, m, w_qkv, w_out)
    res = run_bass_kernel_spmd(nc, in_maps, core_ids=list(range(8)), trace=trace)

    y = np.empty((B, T, D), np.float32)
    for b in range(B):
        y[b] = res.results[2 * b]["y"] + res.results[2 * b + 1]["y"]
    y += b_out[None, None, :]
    y *= m[..., None]
    return y, res


def kernel(**inputs) -> np.ndarray:
    y, _ = _execute(inputs, trace=False)
    return y


# revision 28
# speedup vs baseline: 1.2478x; 1.0039x over previous
"""Trainium2 Bass kernel for nn_DiffusionModel_56822417326086.

Causal multi-head self-attention block:
    qkv = x @ w_qkv ; split into 8 heads of 64
    e = (q @ k^T) * DH^-0.5 ; causal + key-padding mask ; a = softmax(e)
    o = a @ v ; y = o @ w_out + b_out ; y *= m

Sharding (8 cores, zero collectives):
    core c -> batch b = c // 2, head-quad q = c % 2 (heads 4q..4q+3).
    Each core computes q/k/v for its 4 heads over its whole batch, full
    causal attention for those heads, and the partial output projection
    y_partial = o[heads] @ w_out[head rows].  Host sums the two partials
    per batch (linear unshard), adds b_out, applies the query-side mask.

On-device layout notes:
  - scores are computed TRANSPOSED: sT[key, query] so that the A@V
    contraction (over keys) has keys on the partition dim.
  - softmax denominators come for free as a 65th "ones" column of V.
  - no max-subtraction in softmax: scores are O(1) here, exp is safe.
  - matmuls run as float32r (fp32 data on the fast PE path).
  - all matmul operands live at partition base 0 (base-64 operands fault
    on this runtime), so q/k are stored per-head at partitions 0-63.
  - all 4 heads of one key block share a 2-bank PSUM tile [128, 1024]
    so one ACT Exp op covers them (ACT per-op overhead is ~250 ns).
"""

import numpy as np
import ml_dtypes
from contextlib import ExitStack

B, T, D, H = 4, 2048, 512, 8
DH = D // H
SCALE = DH ** -0.5
NEG = -1.0e30
QC = 512           # query-chunk (free dim of score matmuls)
NQC = T // QC      # 8
KB = 128           # key-block (partition dim of score tiles)

_CACHE = {}


def _build_program():
    import concourse.mybir as mybir
    import concourse.tile as tile
    from concourse import bacc

    f32 = mybir.dt.float32
    f32r = mybir.dt.float32r
    bf16 = mybir.dt.bfloat16
    Exp = mybir.ActivationFunctionType.Exp

    nc = bacc.Bacc("TRN2", target_bir_lowering=False, debug=False)

    xT_d = nc.dram_tensor("xT", [D, T], bf16, kind="ExternalInput").ap()
    wq_d = nc.dram_tensor("wq2", [2, D, 128], bf16, kind="ExternalInput").ap()
    wk_d = nc.dram_tensor("wk2", [2, D, 128], bf16, kind="ExternalInput").ap()
    wv_d = nc.dram_tensor("wv4", [D, 256], bf16, kind="ExternalInput").ap()
    wo_d = nc.dram_tensor("wo4", [256, D], f32r, kind="ExternalInput").ap()
    dm_d = nc.dram_tensor("dm4", [4, 128, 1024], bf16, kind="ExternalInput").ap()
    mk_d = nc.dram_tensor("mkey", [T, 1], f32, kind="ExternalInput").ap()
    y_d = nc.dram_tensor("y", [T, D], f32, kind="ExternalOutput").ap()

    with tile.TileContext(nc) as tc, ExitStack() as ctx:
        consts = ctx.enter_context(tc.tile_pool(name="consts", bufs=1))
        work = ctx.enter_context(tc.tile_pool(name="work", bufs=2))
        ps_big = ctx.enter_context(tc.tile_pool(name="psb", bufs=3, space="PSUM"))
        ps_o = ctx.enter_context(tc.tile_pool(name="pso", bufs=1, space="PSUM"))

        # ---- persistent tiles ----------------------------------------------
        # packed q^T/k^T: partitions 0-63 = head A of pair, 64-127 = head B
        qT2 = consts.tile([128, 2, T], f32r)
        kT2 = consts.tile([128, 2, T], f32r)
        vsb = consts.tile([128, 16, 4, 65], bf16)
        wo = consts.tile([128, 2, D], f32r)
        mk = consts.tile([128, 16], f32)
        ones41 = consts.tile([128, 4, 1], f32)
        oUA = consts.tile([64, 2, T], f32)
        oUB = consts.tile([64, 2, T], f32)
        sums_stage = consts.tile([36, 1024], f32)   # p0 rows 0-3, p1 rows 32-35
        recips_f = consts.tile([36, 1024], f32)
        recips = consts.tile([36, 1024], f32r)
        ones64 = consts.tile([1, 64], f32)
        ones64r = consts.tile([1, 64], f32r)
        oTn2 = consts.tile([128, 2, T], f32r)

        nc.vector.memset(ones41[:], 1.0)
        nc.vector.memset(ones64[:], 1.0)
        nc.vector.tensor_copy(ones64r[:], ones64[:])
        warm = consts.tile([1, 512], f32r)
        nc.vector.tensor_copy(warm[0:1, 0:64], ones64[:])
        for _ in range(40):
            wps = ps_big.tile([64, 512], f32, tag="scores")
            nc.tensor.matmul(wps[:], ones64r[:], warm[:], start=True, stop=True)
        for p in range(2):
            nc.sync.dma_start(wo[:, p, :], wo_d[p * 128:(p + 1) * 128, :])
        for rc in range(16):
            nc.sync.dma_start(mk[:, rc:rc + 1], mk_d[rc * 128:(rc + 1) * 128, :])

        # ---- qkv projection (phase-scoped SBUF pool) ------------------------
        with tc.tile_pool(name="qkvp", bufs=1) as qp:
            wq = qp.tile([128, 2, 4, 128], bf16)
            wk = qp.tile([128, 2, 4, 128], bf16)
            wv = qp.tile([128, 4, 256], bf16)
            for p in range(2):
                for kc in range(4):
                    nc.gpsimd.dma_start(wq[:, p, kc, :],
                                        wq_d[p, kc * 128:(kc + 1) * 128, :])
                    nc.scalar.dma_start(wk[:, p, kc, :],
                                        wk_d[p, kc * 128:(kc + 1) * 128, :])
            for kc in range(4):
                nc.sync.dma_start(wv[:, kc, :], wv_d[kc * 128:(kc + 1) * 128, :])
            xT = qp.tile([128, 4, T], bf16)
            # column-major sub-chunks so the first matmul group's inputs land
            # quickly instead of after the whole 4 MB of x
            _eng = [nc.sync, nc.gpsimd, nc.scalar, nc.gpsimd]
            for rc4 in range(4):
                for kc in range(4):
                    _eng[kc].dma_start(
                        xT[:, kc, rc4 * 512:(rc4 + 1) * 512],
                        xT_d[kc * 128:(kc + 1) * 128, rc4 * 512:(rc4 + 1) * 512])

            for p in range(2):
                for rc4 in range(4):
                    sl = slice(rc4 * 512, (rc4 + 1) * 512)
                    psq = ps_big.tile([128, 512], f32, tag="scores")
                    psk = ps_big.tile([128, 512], f32, tag="scores")
                    for kc in range(4):
                        nc.tensor.matmul(psq[:], wq[:, p, kc, :], xT[:, kc, sl],
                                         start=kc == 0, stop=kc == 3)
                        nc.tensor.matmul(psk[:], wk[:, p, kc, :], xT[:, kc, sl],
                                         start=kc == 0, stop=kc == 3)
                    nc.vector.tensor_copy(qT2[:, p, sl], psq[:])
                    nc.vector.tensor_copy(kT2[:, p, sl], psk[:])

            for rc in range(16):
                psv = ps_big.tile([128, 4, 64], f32, tag="scores")
                for kc in range(4):
                    nc.tensor.matmul(psv[:], xT[:, kc, rc * 128:(rc + 1) * 128],
                                     wv[:, kc, :], start=kc == 0, stop=kc == 3)
                nc.vector.tensor_scalar_mul(vsb[:, rc, :, 0:64], psv[:],
                                            mk[:, rc:rc + 1])
                nc.vector.tensor_scalar_mul(vsb[:, rc, :, 64:65], ones41[:],
                                            mk[:, rc:rc + 1])

        # ---- attention (pair-major; 2 heads per 2-bank score tile) ----------
        with tc.tile_pool(name="attp", bufs=1) as ap_, \
             tc.tile_pool(name="exp", bufs=4) as exp_pool:
            dm = ap_.tile([128, 4, 1024], bf16)
            for v_ in range(4):
                nc.sync.dma_start(dm[:, v_, :], dm_d[v_])
            for p in range(2):
                for qc in range(NQC):
                    nkb = 4 * (qc + 1)
                    qsl = slice(qc * QC, (qc + 1) * QC)
                    oA = ps_o.tile([128, 512], f32, tag="oA")
                    oB = ps_o.tile([128, 512], f32, tag="oB")
                    avq = []
                    for kb in range(nkb):
                        ksl = slice(kb * KB, (kb + 1) * KB)
                        sps = ps_big.tile([128, 1024], f32, tag="scores")
                        # row-tiled pair: K=64 each, concurrent in the array;
                        # outputs land in DIFFERENT PSUM banks (same-bank
                        # dual-write faults the exec unit)
                        nc.tensor.matmul(sps[:, 0:512], kT2[0:64, p, ksl],
                                         qT2[0:64, p, qsl], start=True, stop=True,
                                         tile_position=(0, 0))
                        nc.tensor.matmul(sps[:, 512:1024], kT2[64:128, p, ksl],
                                         qT2[64:128, p, qsl], start=True, stop=True,
                                         tile_position=(64, 0))
                        ex = exp_pool.tile([128, 1024], bf16, tag="exp")
                        nc.scalar.activation(ex[:], sps[:], Exp, scale=SCALE)
                        if kb >= nkb - 4:
                            nc.vector.tensor_mul(ex[:], ex[:],
                                                 dm[:, kb - (nkb - 4), :])
                        avq.append((kb, ex))
                        if len(avq) > 1:
                            _em(nc, avq.pop(0), oA, oB, vsb, p, nkb)
                    _em(nc, avq.pop(0), oA, oB, vsb, p, nkb)

                    scr = work.tile([128, 1024], f32, tag="sumscr")
                    nc.vector.tensor_copy(scr[64:65, 0:512], oA[64:65, :])
                    nc.vector.tensor_copy(scr[64:65, 512:1024], oB[64:65, :])
                    idx = p * 32 + qc
                    nc.sync.dma_start(sums_stage[idx:idx + 1, :], scr[64:65, :])
                    nc.vector.tensor_copy(oUA[:, p, qsl], oA[0:64, :])
                    nc.vector.tensor_copy(oUB[:, p, qsl], oB[0:64, :])

        # ---- normalize + output projection ---------------------------------
        # reciprocal is split per pair: p0's normalization only depends on
        # p0's sums, so the scheduler can run it under p1's attention.
        for p in range(2):
            nc.vector.reciprocal(recips_f[p * 32:p * 32 + 4, :],
                                 sums_stage[p * 32:p * 32 + 4, :])
            nc.vector.tensor_copy(recips[p * 32:p * 32 + 4, :],
                                  recips_f[p * 32:p * 32 + 4, :])
        for qc in range(NQC):
            qsl = slice(qc * QC, (qc + 1) * QC)
            for p in range(2):
                idx = p * 32 + qc
                rec = work.tile([1, 1024], f32r, tag="rec")
                nc.sync.dma_start(rec[:], recips[idx:idx + 1, :])
                bcA = ps_big.tile([64, 512], f32, tag="scores")
                nc.tensor.matmul(bcA[:], ones64r[:], rec[0:1, 0:512],
                                 start=True, stop=True)
                nc.vector.tensor_mul(oTn2[0:64, p, qsl], oUA[:, p, qsl], bcA[:])
                bcB = ps_big.tile([64, 512], f32, tag="scores")
                nc.tensor.matmul(bcB[:], ones64r[:], rec[0:1, 512:1024],
                                 start=True, stop=True)
                scrB = work.tile([64, 512], f32r, tag="scrB")
                nc.vector.tensor_mul(scrB[:], oUB[:, p, qsl], bcB[:])
                # partition shift 0-63 -> 64-127 (DVE lanes are partition-locked)
                nc.sync.dma_start(oTn2[64:128, p, qsl], scrB[:])
            for rc in range(4 * qc, 4 * qc + 4):
                rsl = slice(rc * 128, (rc + 1) * 128)
                psy = ps_big.tile([128, 512], f32, tag="scores")
                for p in range(2):
                    nc.tensor.matmul(psy[:], oTn2[:, p, rsl], wo[:, p, :],
                                     start=p == 0, stop=p == 1)
                yt = work.tile([128, 512], f32, tag="ysb")
                nc.vector.tensor_copy(yt[:], psy[:])
                nc.sync.dma_start(y_d[rsl, :], yt[:])

    nc.compile()
    return nc


def _em(nc, item, oA, oB, vsb, p, nkb):
    """Emit the deferred A@V accumulations for one key block (one pair)."""
    kb, ex = item
    nc.tensor.matmul(oA[0:65, :], vsb[:, kb, 2 * p, :], ex[:, 0:512],
                     start=kb == 0, stop=kb == nkb - 1)
    nc.tensor.matmul(oB[0:65, :], vsb[:, kb, 2 * p + 1, :], ex[:, 512:1024],
                     start=kb == 0, stop=kb == nkb - 1)


def _diag_masks():
    i = np.arange(QC)[None, :]
    j = np.arange(KB)[:, None]
    out = []
    for v in range(4):
        mv = np.where(i >= j + v * KB, 1.0, 0.0).astype(np.float32)
        out.append(np.tile(mv, (1, 2)).copy())
    return out


def _prep_inputs(x, m, w_qkv, w_out):
    """Per-core input maps for SPMD dispatch."""
    dm4 = np.stack(_diag_masks()).astype(ml_dtypes.bfloat16)
    wq_full = w_qkv[:, 0:D]
    wk_full = w_qkv[:, D:2 * D]
    wv_full = w_qkv[:, 2 * D:3 * D]
    in_maps = []
    for c in range(8):
        b, q = c // 2, c % 2
        hsl = slice(4 * q * DH, (4 * q + 4) * DH)
        wq2 = np.stack([
            np.concatenate([wq_full[:, (4 * q + 2 * p) * DH:(4 * q + 2 * p + 1) * DH],
                            wq_full[:, (4 * q + 2 * p + 1) * DH:(4 * q + 2 * p + 2) * DH]],
                           axis=1)
            for p in range(2)])
        wk2 = np.stack([
            np.concatenate([wk_full[:, (4 * q + 2 * p) * DH:(4 * q + 2 * p + 1) * DH],
                            wk_full[:, (4 * q + 2 * p + 1) * DH:(4 * q + 2 * p + 2) * DH]],
                           axis=1)
            for p in range(2)])
        in_maps.append({
            "xT": np.ascontiguousarray(x[b].T).astype(ml_dtypes.bfloat16),
            "wq2": np.ascontiguousarray(wq2).astype(ml_dtypes.bfloat16),
            "wk2": np.ascontiguousarray(wk2).astype(ml_dtypes.bfloat16),
            "wv4": np.ascontiguousarray(wv_full[:, hsl]).astype(ml_dtypes.bfloat16),
            "wo4": np.ascontiguousarray(w_out[hsl, :]).astype(np.float32),
            "dm4": dm4,
            "mkey": np.ascontiguousarray((m[b] != 0).astype(np.float32)[:, None]),
        })
    return in_maps


def _execute(inputs, trace=False):
    from concourse.bass_utils import run_bass_kernel_spmd

    if "nc" not in _CACHE:
        _CACHE["nc"] = _build_program()
    nc = _CACHE["nc"]

    x = np.asarray(inputs["x"], np.float32)
    m = np.asarray(inputs["m"], np.float32)
    w_qkv = np.asarray(inputs["w_qkv"], np.float32)
    w_out = np.asarray(inputs["w_out"], np.float32)
    b_out = np.asarray(inputs["b_out"], np.float32)

    in_maps = _prep_inputs(x, m, w_qkv, w_out)
    res = run_bass_kernel_spmd(nc, in_maps, core_ids=list(range(8)), trace=trace)

    y = np.empty((B, T, D), np.float32)
    for b in range(B):
        y[b] = res.results[2 * b]["y"] + res.results[2 * b + 1]["y"]
    y += b_out[None, None, :]
    y *= m[..., None]
    return y, res


def kernel(**inputs) -> np.ndarray:
    y, _ = _execute(inputs, trace=False)
    return y
